# revision 33
# baseline (speedup 1.0000x reference)
"""Multi-head self-attention (B=2, S=2048, D=1024, H=16) on 8 Trainium2 NeuronCores.

Sharding: batch x head-group. Core c = b*4 + g handles batch b and heads 4g..4g+3
(Megatron-style TP: Wq/Wk/Wv column-sharded, Wo row-sharded; partial outputs
summed on the host).

v4 design (bf16 compute, fp32 PSUM accumulation), T-layout (sequence on the
free dim everywhere):
  QT/KT = (w.T @ xt) [256, 2048]      d' on partitions
  V     = (xt.T @ wv) [2048, 256]     natural layout
  scoresT[k, q] = KT_h.T @ QT_h       per head, K=64 row pairs (2 heads
                                      concurrent in rows 0-63 / 64-127)
  expT = exp(scoresT / 8)             bf16, scalar engine (no max subtraction:
                                      |scores| <~ 2)
  ctxT[d', q]  col-tiled pair: head e -> PE columns e*64..e*64+63, both heads
               accumulate CONCURRENTLY into one shared PSUM bank (partition-
               disjoint rows, skip_group_check for the conservative sim gate)
  den[q]       ones[128,64] stationary -> 64 duplicated rows per head in a
               second shared bank; the answer comes out pre-broadcast, so
               normalize is one reciprocal + one multiply for BOTH heads
  outT_partial = wo.T @ ctxT          bf16 out, host sums partials

Scheduling: the scalar-engine exp stream (~143us) is the critical path:
  - xs is DMA'd per-ko and the lead-in runs KT-m0 as 4 parallel ko-outer
    PSUM chains + QT-m0-n0, so scores fire as soon as the last chunk lands.
  - loop is head-pair-OUTER: hp=0 blocks only need the m=0 projections; all
    m=1 projections, V chains, and output projections are spread as PE fill
    work into the exp-bound kc loops of later blocks.
"""
import sys

sys.path.insert(0, "/opt/trn_rl_repo")

import numpy as np
import ml_dtypes

import concourse.bass as bass
import concourse.tile as tile
from concourse import bacc, mybir
from concourse.bass_utils import run_bass_kernel_spmd

F32 = mybir.dt.float32
BF16 = mybir.dt.bfloat16

S = 2048          # sequence length per batch
D = 1024          # embedding dim
HG = 4            # heads per core
HD = 64           # head dim
GC = HG * HD      # group cols = 256
P = 128
NQ = 4            # q chunks of 512
QW = 512          # q chunk width
NKC = 16          # key-position chunks of 128
KO = 8            # contraction chunks of 128 over D

_NC_CACHE = {}


def _build():
    if "nc" in _NC_CACHE:
        return _NC_CACHE["nc"]
    nc = bacc.Bacc(trn_type="TRN2", target_bir_lowering=False, debug=False)
    # weights arrive host-swizzled to the SBUF layout so each DMA is one
    # contiguous 4KB-per-partition stream (512B packets otherwise gate the
    # first matmul on a slow scattered transfer)
    xt_d = nc.dram_tensor("xt", [D, S], BF16, kind="ExternalInput")
    wq_d = nc.dram_tensor("wq", [P, KO * GC], BF16, kind="ExternalInput")
    wk_d = nc.dram_tensor("wk", [P, KO * GC], BF16, kind="ExternalInput")
    wv_d = nc.dram_tensor("wv", [P, KO * GC], BF16, kind="ExternalInput")
    wo_d = nc.dram_tensor("wo", [P, 2 * D], BF16, kind="ExternalInput")
    out_d = nc.dram_tensor("out_t", [D, S], BF16, kind="ExternalOutput")
    with tile.TileContext(nc) as tc:
        _emit(nc, tc, xt_d, wq_d, wk_d, wv_d, wo_d, out_d)
    nc.compile()
    _NC_CACHE["nc"] = nc
    return nc


def _emit(nc, tc, xt_d, wq_d, wk_d, wv_d, wo_d, out_d):
    with tc.tile_pool(name="big", bufs=1) as big, \
         tc.tile_pool(name="expp", bufs=6) as expp, \
         tc.tile_pool(name="norm", bufs=2) as norm, \
         tc.tile_pool(name="evac", bufs=2) as evac, \
         tc.tile_pool(name="outp", bufs=3) as outp, \
         tc.tile_pool(name="ps_sc", bufs=2, space="PSUM") as ps_sc, \
         tc.tile_pool(name="ps_ctx", bufs=1, space="PSUM") as ps_ctx, \
         tc.tile_pool(name="ps_o", bufs=2, space="PSUM") as ps_o:
        # ---- persistent SBUF tensors (~75KB/partition, bf16) ----
        xs = big.tile([P, KO, S], BF16)         # x.T, [d_in(128) x ko x s]
        wqs = big.tile([P, KO, GC], BF16)
        wks = big.tile([P, KO, GC], BF16)
        wvs = big.tile([P, KO, GC], BF16)
        wos = big.tile([P, 2, D], BF16)         # [d'(128) x chunk x e]
        qt = big.tile([P, 2, S], BF16)          # head h at parts (h%2)*64, chunk h//2
        kt = big.tile([P, 2, S], BF16)
        va = big.tile([P, NKC, GC], BF16)       # V natural
        ct = big.tile([P, 2, S], BF16)          # ctxT, same head layout as qt
        ones = big.tile([P, HD], BF16)          # stationary for den matmuls

        # DMA issue spread across engine queues; xs per-ko so the ko-outer
        # lead-in chains start after the first chunk, not the whole tensor.
        nc.scalar.dma_start(wks[:].rearrange("p ko m -> p (ko m)"), wk_d[:])
        nc.scalar.dma_start(wqs[:].rearrange("p ko m -> p (ko m)"), wq_d[:])
        xt_r = xt_d.rearrange("(ko p) s -> p ko s", p=P)
        for ko in range(KO):
            nc.sync.dma_start(xs[:, ko, :], xt_r[:, ko, :])
        nc.scalar.dma_start(wvs[:].rearrange("p ko m -> p (ko m)"), wv_d[:])
        nc.scalar.dma_start(wos[:].rearrange("p c e -> p (c e)"), wo_d[:])

        nc.vector.memset(ones[:].bitcast(mybir.dt.uint16), 0x3F80)

        def proj_chain(w_sb, m, n, dst):
            """dst[:, m, n*QW:] = sum_ko w_sb[:,ko,m*128:+128].T @ xs[:,ko,nq]"""
            pp = ps_o.tile([P, QW], F32, tag="po", name=f"pj_{m}_{n}")
            for ko in range(KO):
                nc.tensor.matmul(pp[:], w_sb[:, ko, m * P:(m + 1) * P],
                                 xs[:, ko, n * QW:(n + 1) * QW],
                                 start=(ko == 0), stop=(ko == KO - 1))
            nc.vector.tensor_copy(dst[:, m, n * QW:(n + 1) * QW], pp[:])

        def v_chain(kc):
            """va[:, kc, :] = xs[:, :, kc-chunk].T @ wv  (natural V)"""
            pv = ps_o.tile([P, QW], F32, tag="po", name=f"pv_{kc}")
            for ko in range(KO):
                nc.tensor.matmul(pv[:, 0:GC], xs[:, ko, kc * P:(kc + 1) * P],
                                 wvs[:, ko, :],
                                 start=(ko == 0), stop=(ko == KO - 1))
            nc.vector.tensor_copy(va[:, kc, :], pv[:, 0:GC])

        def po_evac(pp, mo, n, tail=False):
            # at the tail the scalar engine is idle: split evac+DMA across
            # scalar and vector so the last 8 outputs drain in parallel
            ot = outp.tile([P, QW], BF16, tag="ot")
            if tail and mo % 2 == 1:
                nc.scalar.copy(ot[:], pp)
                nc.scalar.dma_start(
                    out_d[mo * P:(mo + 1) * P, n * QW:(n + 1) * QW], ot[:])
            else:
                nc.vector.tensor_copy(ot[:], pp)
                nc.sync.dma_start(
                    out_d[mo * P:(mo + 1) * P, n * QW:(n + 1) * QW], ot[:])

        def po_chain(mo, n, tail=False):
            """out_t[mo*128:+128, nq] = sum_c wos[:,c,mo*128:+128].T @ ct[:,c,nq]"""
            pp = ps_o.tile([P, QW], F32, tag="po", name=f"po_{mo}_{n}")
            for c in range(2):
                nc.tensor.matmul(pp[:], wos[:, c, mo * P:(mo + 1) * P],
                                 ct[:, c, n * QW:(n + 1) * QW],
                                 start=(c == 0), stop=(c == 1))
            po_evac(pp[:], mo, n, tail)

        # ---- lead-in: ko-outer KT-m0-n0 + QT-m0-n0 interleaved — the PE pace
        # (2 matmuls/ko, running cold) matches the xs DMA pace, so the first
        # scores fire as soon as the last chunk lands.
        ktp = ps_sc.tile([P, 2, QW], F32, tag="psc", name="lead_k")
        for ko in range(KO):
            nc.tensor.matmul(ktp[:, 0, :], wks[:, ko, 0:P], xs[:, ko, 0:QW],
                             start=(ko == 0), stop=(ko == KO - 1))
            nc.tensor.matmul(ktp[:, 1, :], wqs[:, ko, 0:P], xs[:, ko, 0:QW],
                             start=(ko == 0), stop=(ko == KO - 1))
        nc.vector.tensor_copy(kt[:, 0, 0:QW], ktp[:, 0, :])
        nc.vector.tensor_copy(qt[:, 0, 0:QW], ktp[:, 1, :])

        # ---- fill work: (deadline_kc, thunk) spread into the kc loops ----
        fills = {
            (0, 0): [(0, lambda: proj_chain(wks, 0, 1, kt)),
                     (4, lambda: proj_chain(wks, 0, 2, kt)),
                     (8, lambda: proj_chain(wks, 0, 3, kt)),
                     (10, lambda: proj_chain(wqs, 0, 1, qt))],
            (0, 1): [(3, lambda: proj_chain(wqs, 0, 2, qt)),
                     (9, lambda: proj_chain(wks, 1, 0, kt))],
            (0, 2): [(3, lambda: proj_chain(wqs, 0, 3, qt)),
                     (9, lambda: proj_chain(wks, 1, 1, kt))],
            (0, 3): [(3, lambda: proj_chain(wks, 1, 2, kt)),
                     (8, lambda: proj_chain(wks, 1, 3, kt)),
                     (11, lambda: proj_chain(wqs, 1, 0, qt))],
            (1, 0): [(4, lambda: proj_chain(wqs, 1, 1, qt))],
            (1, 1): [(0, lambda: proj_chain(wqs, 1, 2, qt))]
                    + [(2 + 2 * i, lambda i=i: po_chain(i, 0)) for i in range(8)],
            (1, 2): [(0, lambda: proj_chain(wqs, 1, 3, qt))]
                    + [(2 + 2 * i, lambda i=i: po_chain(i, 1)) for i in range(8)],
            (1, 3): [(2 * i, lambda i=i: po_chain(i, 2)) for i in range(8)],
        }

        # ---- main loop: head-pair hp OUTER, q-block n inner ----
        def emit_scores(hp, n, kc, sps):
            sp = ps_sc.tile([P, 2, QW], F32, tag="psc", name=f"sp_{hp}_{n}_{kc}")
            for e in range(2):   # head 2hp+e in rows e*64..e*64+63
                lo = e * HD
                nc.tensor.matmul(
                    sp[:, e, :],
                    kt[lo:lo + HD, hp, kc * P:(kc + 1) * P],
                    qt[lo:lo + HD, hp, n * QW:(n + 1) * QW],
                    start=True, stop=True)
            sps[kc] = sp

        blocks = [(hp, n) for hp in range(2) for n in range(NQ)]
        sps_carry = {}
        for bi, (hp, n) in enumerate(blocks):
            if True:
                fill = sorted(fills.get((hp, n), []), key=lambda t: t[0])
                fi = 0
                # both heads share one ctx bank (partition-disjoint col tiles)
                cps = ps_ctx.tile([P, QW], F32, tag="pc", name=f"pc_{hp}_{n}")
                dps = ps_ctx.tile([P, QW], F32, tag="pd", name=f"pd_{hp}_{n}")
                sps = sps_carry
                sps_carry = {}
                if bi == 0:
                    emit_scores(hp, n, 0, sps)
                    emit_scores(hp, n, 1, sps)
                for kc in range(NKC):
                    # exp first so the scalar engine never starves, then the
                    # next scores (to refill its pipeline), then fill work,
                    # then this kc's ctx/den consumers. The last two slots
                    # pre-emit the NEXT block's first scores so the exp
                    # stream crosses block boundaries without a gap.
                    sp = sps.pop(kc)
                    ex = expp.tile([P, 2, QW], BF16, tag="pex")
                    nc.scalar.activation(
                        ex[:].rearrange("p a b -> p (a b)"),
                        sp[:].rearrange("p a b -> p (a b)"),
                        mybir.ActivationFunctionType.Exp,
                        scale=0.125)
                    if kc + 2 < NKC:
                        emit_scores(hp, n, kc + 2, sps)
                    elif bi + 1 < len(blocks):
                        nhp, nn = blocks[bi + 1]
                        emit_scores(nhp, nn, kc + 2 - NKC, sps_carry)
                    if hp == 0 and n == 0:
                        v_chain(kc)          # va[kc] needed by ctx(kc) below
                    if fi < len(fill) and fill[fi][0] <= kc:
                        fill[fi][1]()
                        fi += 1
                    for e in range(2):   # col-tiled pair, concurrent heads
                        h = 2 * hp + e
                        nc.tensor.matmul(
                            cps[e * HD:(e + 1) * HD, :],
                            va[:, kc, h * HD:(h + 1) * HD],
                            ex[:, e, :],
                            start=(kc == 0), stop=(kc == NKC - 1),
                            skip_group_check=True)
                    for e in range(2):   # denominators, pre-broadcast 64 rows
                        nc.tensor.matmul(
                            dps[e * HD:(e + 1) * HD, :],
                            ones[:], ex[:, e, :],
                            start=(kc == 0), stop=(kc == NKC - 1),
                            skip_group_check=True)
                while fi < len(fill):
                    fill[fi][1]()
                    fi += 1
                # normalize both heads at once: ct = cps * (1 / dps)
                dsb = norm.tile([P, QW], F32, tag="nd")
                nc.vector.tensor_copy(dsb[:], dps[:])
                rr = norm.tile([P, QW], F32, tag="nr")
                nc.vector.reciprocal_approx_fast(rr[:], dsb[:])
                nc.vector.tensor_tensor(
                    ct[:, hp, n * QW:(n + 1) * QW],
                    cps[:], rr[:], mybir.AluOpType.mult)
        # ---- tail: output projection of the last q-block, widened across
        # the now-idle scores PSUM banks so the chains pipeline.
        tp = ps_sc.tile([P, 2, QW], F32, tag="psc", name="tail_a")
        tp2 = ps_sc.tile([P, 2, QW], F32, tag="psc", name="tail_b")
        tails = [tp[:, 0, :], tp[:, 1, :], tp2[:, 0, :], tp2[:, 1, :]]
        for mo in range(KO):
            if mo < 4:
                pp = tails[mo]
                for c in range(2):
                    nc.tensor.matmul(pp, wos[:, c, mo * P:(mo + 1) * P],
                                     ct[:, c, (NQ - 1) * QW:NQ * QW],
                                     start=(c == 0), stop=(c == 1))
                po_evac(pp, mo, NQ - 1, tail=True)
            else:
                po_chain(mo, NQ - 1, tail=True)


def _in_maps(x, wq_f, wk_f, wv_f, wo_f):
    bf = ml_dtypes.bfloat16

    def swz(w):  # [1024, 256] -> [128, 8*256] SBUF layout (p, ko, m)
        return np.ascontiguousarray(
            w.reshape(KO, P, GC).transpose(1, 0, 2).reshape(P, KO * GC)).astype(bf)

    maps = []
    for core in range(8):
        b, g = core // 4, core % 4
        cols = slice(g * GC, (g + 1) * GC)
        wo_c = wo_f[cols, :]          # [256, 1024] -> [128, 2*1024] (p, c, e)
        maps.append({
            "xt": np.ascontiguousarray(x[b].T).astype(bf),
            "wq": swz(wq_f[:, cols]),
            "wk": swz(wk_f[:, cols]),
            "wv": swz(wv_f[:, cols]),
            "wo": np.ascontiguousarray(
                wo_c.reshape(2, P, D).transpose(1, 0, 2).reshape(P, 2 * D)).astype(bf),
        })
    return maps


def _prep(x, Wq, Wk, Wv, Wo, q_scale, k_scale, v_scale, o_scale):
    x = np.asarray(x, dtype=np.float32)
    wq_f = (np.asarray(Wq).T * np.asarray(q_scale).reshape(1, -1)).astype(np.float32)
    wk_f = (np.asarray(Wk).T * np.asarray(k_scale).reshape(1, -1)).astype(np.float32)
    wv_f = (np.asarray(Wv).T * np.asarray(v_scale).reshape(1, -1)).astype(np.float32)
    wo_f = (np.asarray(Wo).T * np.asarray(o_scale).reshape(1, -1)).astype(np.float32)
    return x, wq_f, wk_f, wv_f, wo_f


def _gather(res, B):
    out = np.zeros((B, S, D), dtype=np.float32)
    for core in range(8):
        out[core // 4] += res.results[core]["out_t"].astype(np.float32).T
    return out


def run_traced(x, Wq, Wk, Wv, Wo, q_scale, k_scale, v_scale, o_scale):
    """Like kernel() but with NTFF tracing; returns (out, exec_time_ns, trace_path)."""
    x, wq_f, wk_f, wv_f, wo_f = _prep(x, Wq, Wk, Wv, Wo,
                                      q_scale, k_scale, v_scale, o_scale)
    nc = _build()
    res = run_bass_kernel_spmd(nc, _in_maps(x, wq_f, wk_f, wv_f, wo_f),
                               core_ids=list(range(8)), trace=True)
    out = _gather(res, x.shape[0])
    trace_path = None
    if res.instructions_and_trace is not None:
        trace_path = res.instructions_and_trace[1]
    return out, res.exec_time_ns, trace_path


def kernel(x, Wq, Wk, Wv, Wo, q_scale, k_scale, v_scale, o_scale):
    B = x.shape[0]
    x, wq_f, wk_f, wv_f, wo_f = _prep(x, Wq, Wk, Wv, Wo,
                                      q_scale, k_scale, v_scale, o_scale)
    nc = _build()
    res = run_bass_kernel_spmd(nc, _in_maps(x, wq_f, wk_f, wv_f, wo_f),
                               core_ids=list(range(8)))
    return _gather(res, B)
